# revision 2
# baseline (speedup 1.0000x reference)
"""Trainium2 Bass kernel for nn_MAE_65025804861607 — static-instruction-count
minimized rewrite.

On this runtime each STATIC instruction costs ~65us per execution while For_i
loop iterations are nearly free. The kernel therefore packs both images per
core along the free dimension (pitch 4608 = 4096 + 512 gap) and rolls every
chunked stage into For_i hardware loops with ds() dynamic offsets:
  stats loop: ones-matmuls produce sum(x)/sum(x^2) already replicated over
    64 partitions -> LN math is 9 full-width ops, no broadcasts
  proj loop: dt | B|C matmuls (gamma folded host-side)
  conv branch: 9-tap shifted matmuls (tap pairs K=128), 18 trips covering
    both images in one padded tile; boundary-trip damage repaired by two
    border memsets
  scan per group: 16->128 pyramid-replication DMAs; gapped double-image
    tensor_tensor_scan (zeroed gap columns force the h-state reset between
    images); dBx = dt*B*xnc built from replicated tiles; C-multiply in place
  out-proj: per-group For_i accumulating into y via scalar_tensor_tensor,
    Dp-term matmul merged into the last group's accumulation group
  CA gate + final combine; bf16 staging, output DMA casts to f32 on gpsimd.
"""
import sys
import numpy as np

sys.path.insert(0, '/opt/trn_rl_repo')

import concourse.bass as bass
import concourse.mybir as mybir
from concourse.tile import TileContext
from concourse.bass_utils import run_bass_kernel_spmd

F32 = mybir.dt.float32
BF16 = mybir.dt.bfloat16
AF = mybir.ActivationFunctionType
OP = mybir.AluOpType
ds = bass.ds

NCORES = 8
IPC = 2                 # images per core
C = 64
H = W = 64
L = H * W               # 4096
NG, DG, DSTATE = 4, 16, 8
GAP = 512
PIT = L + GAP           # 4608 = 9*512
F = IPC * PIT           # 9216
T = F // 512            # 18 trips
WP = 66                 # padded row width
IMGROWS = 72            # conv image pitch in rows (72*66 = 4752 = 9*528)
CROWS = 148
CW = CROWS * WP         # 9768
VLEN = 145 * WP         # window-view length

_CACHE = {}


def _fix_waits_json(data):
    lim = {"EventSemaphore": 2}
    for fn in data.get("functions", []):
        for blk in fn.get("blocks", []):
            out = []
            for ins in blk.get("instructions", []):
                si = ins.get("sync_info")
                ow = (si or {}).get("on_wait") or []
                limit = lim.get(ins.get("opcode"), 1)
                if len(ow) > limit:
                    excess = ow[: len(ow) - limit]
                    si["on_wait"] = ow[len(ow) - limit:]
                    for k, wv in enumerate(excess):
                        out.append({
                            "debug": ins.get("debug", 0),
                            "engine": ins["engine"],
                            "ins": [], "outs": [],
                            "name": f"{ins['name']}_xw{k}",
                            "opcode": "EventSemaphore",
                            "sync_info": {"on_update": [], "on_wait": [wv]},
                        })
                out.append(ins)
            blk["instructions"] = out
    return data


def _patch_bass_class():
    import json as _json
    cls = bass.Bass
    if getattr(cls, "_waitfix_patched", False):
        return
    orig = cls.to_json_bytes

    def patched(self, *a, **kw):
        data = _json.loads(orig(self, *a, **kw))
        _fix_waits_json(data)
        return _json.dumps(data).encode()

    cls.to_json_bytes = patched
    cls._waitfix_patched = True


def _make_consts(inp):
    qkv_w = inp['qkv_w'][:, :, 0, 0, 0].astype(np.float64)      # (192, 64)
    dw_mid = inp['dw_w'][:, 0, 1, :, :].astype(np.float64)      # (192, 3, 3)
    fc_w = inp['fc_w'][:, :, 0, 0, 0].astype(np.float64)        # (9, 24)
    fc_b = inp['fc_b'].astype(np.float64)
    dep_mid = inp['dep_w'][:, :, 1, :, :].astype(np.float64)    # (64, 9, 3, 3)
    dep_b = inp['dep_b'].astype(np.float64)
    ln_g = inp['ln_g'].astype(np.float64)
    ln_b = inp['ln_b'].astype(np.float64)
    A = -np.exp(inp['A_log'].astype(np.float64))                # (NG, DG, DSTATE)
    Wdt, bdt = inp['Wdt'].astype(np.float64), inp['bdt'].astype(np.float64)
    WB, WC = inp['WB'].astype(np.float64), inp['WC'].astype(np.float64)
    Dp = inp['Dp'].astype(np.float64)
    out_w, out_b = inp['out_w'].astype(np.float64), inp['out_b'].astype(np.float64)

    c = {}
    FCbd = np.zeros((72, 192))
    for d in range(8):
        for o in range(9):
            for k in range(24):
                FCbd[d * 9 + o, k * 8 + d] = fc_w[o, k]
    wtap = np.zeros((9, 64, 72))
    for ty in range(3):
        for tx in range(3):
            k = ty * 3 + tx
            wtap[k] = (FCbd @ (dw_mid[:, ty, tx][:, None] * qkv_w)).T
    wpair = np.zeros((128, 3 * 72), np.float32)
    wsing = np.zeros((64, 3 * 72), np.float32)
    for ty in range(3):
        wpair[0:64, ty * 72:(ty + 1) * 72] = wtap[ty * 3 + 0]
        wpair[64:128, ty * 72:(ty + 1) * 72] = wtap[ty * 3 + 1]
        wsing[:, ty * 72:(ty + 1) * 72] = wtap[ty * 3 + 2]
    c['wpair'] = wpair
    c['wsing'] = wsing
    f2b = np.zeros((72, 1), np.float32)
    for d in range(8):
        for o in range(9):
            f2b[d * 9 + o, 0] = fc_b[o]
    c['f2b'] = f2b
    bdep = np.zeros((72, 9 * 64), np.float32)
    for ty in range(3):
        for tx in range(3):
            k = ty * 3 + tx
            Bt = np.zeros((64, 72))
            for g in range(8):
                Bt[8 * g:8 * g + 8, 9 * g:9 * g + 9] = \
                    dep_mid[8 * g:8 * g + 8, :, ty, tx]
            bdep[:, 64 * k:64 * k + 64] = Bt.T
    c['bdep'] = bdep
    c['depb'] = dep_b.reshape(64, 1).astype(np.float32)

    # stats lhsT [128,128]: cols 0-63 ones over rows 0-63 (sum x, replicated),
    # cols 64-127 ones over rows 64-127 (sum x^2, replicated)
    s2 = np.zeros((128, 128), np.float32)
    s2[0:64, 0:64] = 1.0
    s2[64:128, 64:128] = 1.0
    c['s2lhsT'] = s2

    dtw = np.zeros((64, 64))
    bcw = np.zeros((64, 64))
    bdt_eff = np.zeros((64, 1))
    fbc = np.zeros((64, 1))
    for g in range(NG):
        rows = slice(g * DG, (g + 1) * DG)
        gam = ln_g[rows][:, None]
        bet = ln_b[rows]
        dtw[rows, g * DG:(g + 1) * DG] = Wdt[g] * gam
        bcw[rows, g * 8:g * 8 + 8] = WB[g] * gam
        bcw[rows, 32 + g * 8:32 + g * 8 + 8] = WC[g] * gam
        bdt_eff[g * DG:(g + 1) * DG, 0] = bdt[g] + Wdt[g].T @ bet
        fbc[g * 8:g * 8 + 8, 0] = WB[g].T @ bet
        fbc[32 + g * 8:32 + g * 8 + 8, 0] = WC[g].T @ bet
    c['dtw'] = dtw.astype(np.float32)
    c['bcw'] = bcw.astype(np.float32)
    c['bdt'] = bdt_eff.astype(np.float32)
    c['fbc'] = fbc.astype(np.float32)
    c['ones'] = np.ones((64, 1), np.float32)
    c['eps'] = np.full((64, 1), 1e-5, np.float32)
    c['gam'] = ln_g.reshape(64, 1).astype(np.float32)
    c['bet'] = ln_b.reshape(64, 1).astype(np.float32)

    app = np.zeros((128, NG), np.float32)
    for g in range(NG):
        for n in range(DSTATE):
            for d in range(DG):
                app[n * 16 + d, g] = A[g, d, n]
    c['app'] = app
    opw = np.zeros((128, NG * 64), np.float32)
    for g in range(NG):
        for n in range(DSTATE):
            for d in range(DG):
                opw[n * 16 + d, g * 64:(g + 1) * 64] = out_w[:, g * DG + d]
    c['opw'] = opw
    c['dpw'] = (out_w * Dp.reshape(-1)[None, :]).T.astype(np.float32)
    c['outb'] = out_b.reshape(64, 1).astype(np.float32)
    c['ca1w'] = (inp['ca_w1'].astype(np.float64).T / L).astype(np.float32)
    c['ca1b'] = inp['ca_b1'].reshape(16, 1).astype(np.float32)
    c['ca2w'] = inp['ca_w2'].astype(np.float64).T.astype(np.float32)
    c['ca2bn'] = -inp['ca_b2'].reshape(64, 1).astype(np.float32)
    return c


CONST_SPECS = [
    ('wpair', [128, 3 * 72], BF16), ('wsing', [64, 3 * 72], BF16),
    ('f2b', [72, 1], F32), ('bdep', [72, 9 * 64], BF16),
    ('depb', [64, 1], F32), ('s2lhsT', [128, 128], BF16),
    ('dtw', [64, 64], BF16), ('bcw', [64, 64], BF16),
    ('bdt', [64, 1], F32), ('fbc', [64, 1], F32), ('ones', [64, 1], F32),
    ('eps', [64, 1], F32),
    ('gam', [64, 1], F32), ('bet', [64, 1], F32),
    ('app', [128, NG], F32), ('opw', [128, NG * 64], BF16),
    ('dpw', [64, 64], BF16), ('outb', [64, 1], F32),
    ('ca1w', [64, 16], BF16), ('ca1b', [16, 1], F32),
    ('ca2w', [16, 64], BF16), ('ca2bn', [64, 1], F32),
]


def _build(reps=1):
    _patch_bass_class()
    nc = bass.Bass("TRN2")
    xin = nc.declare_dram_parameter("x", [IPC, C, H, W], F32, isOutput=False)
    out = nc.declare_dram_parameter("out", [IPC, C, H, W], F32, isOutput=True)
    dram = {n: nc.declare_dram_parameter(n, s, F32, isOutput=False)
            for n, s, _ in CONST_SPECS}

    xin_f = xin.rearrange("i c h w -> (i c) (h w)")
    out_f = out.rearrange("i c h w -> (i c) (h w)")

    def gv(t, p0, p1):
        """gap-skipping view rows p0:p1: [p, i, 0:L]"""
        return t[p0:p1, 0:F].rearrange("p (i c) -> p i c", i=IPC)[:, :, 0:L]

    with TileContext(nc) as tc:
        with tc.tile_pool(name="const", bufs=1) as kpool, \
             tc.tile_pool(name="pers", bufs=1) as pp, \
             tc.tile_pool(name="ps", bufs=1, space="PSUM") as psp:

            kt = {}
            for name, shape, dt in CONST_SPECS:
                kt[name] = kpool.tile(shape, dt, tag=name, name=name)
                eng = nc.gpsimd if dt == BF16 else nc.sync
                eng.dma_start(kt[name][:], dram[name][:])

            # ------- persistent tiles (all big ones bf16) -------
            xx2 = pp.tile([128, F], BF16, tag="xx2", name="xx2")
            xpad = pp.tile([128, CW], BF16, tag="xpad", name="xpad")
            f2p = pp.tile([128, CW], BF16, tag="f2p", name="f2p")
            s1b = pp.tile([128, F], BF16, tag="s1b", name="s1b")   # -> dA
            s2b = pp.tile([128, F], BF16, tag="s2b", name="s2b")   # -> dBx
            xhat = pp.tile([64, F], BF16, tag="xhat", name="xhat")  # -> y
            murb = pp.tile([64, F], BF16, tag="murb", name="murb")  # xnc
            dt_t = pp.tile([64, F], BF16, tag="dt_t", name="dt_t")
            bc_t = pp.tile([64, F], BF16, tag="bc_t", name="bc_t")
            t2 = pp.tile([128, F], BF16, tag="t2", name="t2")
            ymean = pp.tile([64, 2], BF16, tag="ymean", name="ymean")
            ca = pp.tile([64, 2], F32, tag="ca", name="ca")
            ca1s = pp.tile([16, 2], BF16, tag="ca1s", name="ca1s")

            ps_dt = psp.tile([64, 512], F32, tag="ps_dt", name="ps_dt")
            ps_bc = psp.tile([64, 512], F32, tag="ps_bc", name="ps_bc")
            ps_f2 = psp.tile([72, 512], F32, tag="ps_f2", name="ps_f2")
            ps_oc = psp.tile([64, 512], F32, tag="ps_oc", name="ps_oc")
            ps_y = psp.tile([64, 512], F32, tag="ps_y", name="ps_y")
            ps_ca1 = psp.tile([16, 2], F32, tag="ps_ca1", name="ps_ca1")
            ps_ca2 = psp.tile([64, 2], F32, tag="ps_ca2", name="ps_ca2")

            # first-rep hygiene: gap columns of xx2 are read by the stats
            # loop before anything writes them
            nc.vector.memset(xx2[:], 0.0)

            for _rep in range(reps):
                # ---- loads: x (and x copy for squares) + padded x ----
                for half in range(2):
                    nc.gpsimd.dma_start(
                        gv(xx2, half * 64, (half + 1) * 64),
                        xin_f[:, :].rearrange("(i c) l -> c i l", i=IPC))
                nc.scalar.activation(xx2[64:128, :], xx2[64:128, :], AF.Square)
                nc.vector.memset(xpad[:], 0.0)
                xpv = xpad[:, :].rearrange("p (h w) -> p h w", w=WP)
                for i in range(IPC):
                    r0 = i * IMGROWS
                    nc.gpsimd.dma_start(
                        xpv[0:64, r0 + 1:r0 + 65, 1:65],
                        xin_f[i * 64:(i + 1) * 64, :]
                        .rearrange("c (h w) -> c h w", h=H))
                    nc.gpsimd.dma_start(
                        xpv[64:128, r0 + 1:r0 + 65, 0:64],
                        xin_f[i * 64:(i + 1) * 64, :]
                        .rearrange("c (h w) -> c h w", h=H))

                # ---- stats loop: sums already replicated over 64 partitions
                for t in range(T):
                    off = t * 512
                    ps_s1 = ps_dt
                    ps_s2 = ps_bc
                    rhs = xx2[:, off:off + 512]
                    nc.tensor.matmul(ps_s1[:], kt['s2lhsT'][:, 0:64],
                                     rhs, start=True, stop=True)
                    nc.tensor.matmul(ps_s2[:], kt['s2lhsT'][:, 64:128],
                                     rhs, start=True, stop=True)
                    nc.scalar.activation(s1b[0:64, off:off + 512], ps_s1[:], AF.Copy)
                    nc.scalar.activation(s2b[0:64, off:off + 512], ps_s2[:], AF.Copy)

                # ---- LN math (rows 0-63, full width) ----
                sc = t2[0:64, :]
                nc.vector.tensor_scalar_mul(s1b[0:64, :], s1b[0:64, :], 1.0 / 64)
                nc.vector.tensor_mul(sc, s1b[0:64, :], s1b[0:64, :])
                nc.vector.scalar_tensor_tensor(s2b[0:64, :], s2b[0:64, :],
                                               1.0 / 64, sc, OP.mult, OP.subtract)
                nc.vector.tensor_scalar_max(s2b[0:64, :], s2b[0:64, :], 1e-6)
                nc.scalar.activation(s2b[0:64, :], s2b[0:64, :], AF.Ln, bias=kt['eps'][:])
                nc.scalar.activation(s2b[0:64, :], s2b[0:64, :], AF.Exp, scale=-0.5)
                nc.vector.tensor_sub(sc, xx2[0:64, :], s1b[0:64, :])
                nc.vector.tensor_mul(xhat[:], sc, s2b[0:64, :])
                nc.vector.tensor_scalar(murb[:], xhat[:], kt['gam'][:],
                                        kt['bet'][:], OP.mult, OP.add)
                xnc = murb

                # ---- proj loop ----
                for t in range(T):
                    off = t * 512
                    rhs = xhat[:, off:off + 512]
                    nc.tensor.matmul(ps_dt[:], kt['dtw'][:],
                                     rhs, start=True, stop=True)
                    nc.tensor.matmul(ps_bc[:], kt['bcw'][:],
                                     rhs, start=True, stop=True)
                    nc.scalar.activation(dt_t[:, off:off + 512], ps_dt[:], AF.Copy)
                    nc.scalar.activation(bc_t[:, off:off + 512], ps_bc[:],
                                         AF.Identity, bias=kt['fbc'][:])
                # softplus: dt = ln(1 + exp(lin + bdt))
                nc.scalar.activation(dt_t[:], dt_t[:], AF.Exp, bias=kt['bdt'][:])
                nc.scalar.activation(dt_t[:], dt_t[:], AF.Ln, bias=kt['ones'][:])

                # ---- conv: f2 ----
                nc.vector.memset(f2p[:], 0.0)
                f2v = f2p[:, :].rearrange("p (h w) -> p h w", w=WP)

                for t in range(T):
                    rb = t * 8
                    for ty in range(3):
                        nc.tensor.matmul(
                            ps_f2[:], kt['wpair'][:, ty * 72:(ty + 1) * 72],
                            xpv[:, rb + ty:rb + ty + 8, 0:64],
                            start=(ty == 0), stop=False)
                    for ty in range(3):
                        nc.tensor.matmul(
                            ps_f2[:], kt['wsing'][:, ty * 72:(ty + 1) * 72],
                            xpv[0:64, rb + ty:rb + ty + 8, 2:66],
                            start=False, stop=(ty == 2))
                    nc.scalar.activation(
                        f2v[0:72, rb + 1:rb + 9, 1:65],
                        ps_f2[:].rearrange("c (a b) -> c a b", a=8),
                        AF.Identity, bias=kt['f2b'][:])
                # repair borders clobbered by the boundary trips (rows 65,137,72)
                nc.vector.memset(f2v[0:72, 65:138:72, :], 0.0)
                nc.vector.memset(f2v[0:72, IMGROWS:IMGROWS + 1, :], 0.0)

                # ---- conv: out_conv accumulated onto x (rows 0-63 of xx2) ----
                for t in range(T):
                    rb = t * 8
                    off = t * 512
                    for k in range(9):
                        ty, tx = k // 3, k % 3
                        nc.tensor.matmul(
                            ps_oc[:], kt['bdep'][:, 64 * k:64 * k + 64],
                            f2v[0:72, rb + ty:rb + ty + 8, tx:tx + 64],
                            start=(k == 0), stop=(k == 8))
                    oco = xx2[0:64, off:off + 512]
                    nc.vector.scalar_tensor_tensor(
                        oco, ps_oc[:], kt['depb'][:], oco, OP.add, OP.add)

                # ---- scan + out-proj per group ----
                dA, dBx = s1b, s2b
                # re-zero gap columns (LN scribbled these tiles)
                for tgap in (dA, dBx):
                    nc.vector.memset(
                        tgap[:, :].rearrange("p (i c) -> p i c", i=IPC)
                        [:, :, L:PIT], 0.0)
                t1 = xpad  # reuse (conv inputs are dead once the scan starts)
                y_t = xhat  # reuse (proj rhs is dead once out-proj starts)

                for g in range(NG):
                    colmajor = g >= 2
                    rev = (g % 2 == 1)

                    def rep128(src_rows):
                        nc.sync.dma_start(t1[0:16, 0:F], src_rows)
                        nc.sync.dma_start(t1[16:32, 0:F], t1[0:16, 0:F])
                        nc.sync.dma_start(t1[32:64, 0:F], t1[0:32, 0:F])
                        nc.sync.dma_start(t1[64:128, 0:F], t1[0:64, 0:F])

                    def cm_out(tile):
                        # scan-order (i, x, y) contiguous view
                        return tile[:, 0:F].rearrange("p (i c) -> p i c", i=IPC) \
                            [:, :, 0:L].rearrange("p i (x y) -> p i x y", x=W)

                    def cm_in(tile):
                        # raster tile read in (i, x, y) order
                        return gv(tile, 0, 128).rearrange(
                            "p i (y x) -> p i y x", y=H).transpose([0, 1, 3, 2])

                    rep128(dt_t[g * 16:(g + 1) * 16, :])
                    if colmajor:
                        nc.scalar.activation(cm_out(dA), cm_in(t1), AF.Exp,
                                             scale=kt['app'][:, g:g + 1])
                    else:
                        nc.scalar.activation(gv(dA, 0, 128), gv(t1, 0, 128),
                                             AF.Exp, scale=kt['app'][:, g:g + 1])
                    # t2 = dt_rep * b_rep  (raster)
                    nc.sync.dma_start(
                        t2[:], bc_t[g * 8:(g + 1) * 8, :]
                        .unsqueeze(1).broadcast_to([8, 16, F]))
                    nc.vector.tensor_tensor(gv(t2, 0, 128), gv(t2, 0, 128),
                                            gv(t1, 0, 128), OP.mult)
                    rep128(xnc[g * 16:(g + 1) * 16, :])
                    if colmajor:
                        nc.vector.tensor_tensor(cm_out(dBx), cm_in(t2),
                                                cm_in(t1), OP.mult)
                    else:
                        nc.vector.tensor_tensor(gv(dBx, 0, 128), gv(t2, 0, 128),
                                                gv(t1, 0, 128), OP.mult)
                    # scan into t1
                    if rev:
                        nc.vector.tensor_tensor_scan(
                            t1[:, 0:F][:, ::-1], dA[:, ::-1], dBx[:, ::-1],
                            0.0, OP.mult, OP.add)
                    else:
                        nc.vector.tensor_tensor_scan(
                            t1[:, 0:F], dA[:], dBx[:], 0.0, OP.mult, OP.add)
                    # c_rep then z = h * c
                    nc.sync.dma_start(
                        t2[:], bc_t[32 + g * 8:32 + (g + 1) * 8, :]
                        .unsqueeze(1).broadcast_to([8, 16, F]))
                    if colmajor:
                        zo = gv(dBx, 0, 128).rearrange("p i (y x) -> p i y x", y=H)
                        hi = t1[:, 0:F].rearrange("p (i c) -> p i c", i=IPC) \
                            [:, :, 0:L].rearrange("p i (x y) -> p i x y", x=W) \
                            .transpose([0, 1, 3, 2])
                        ci = gv(t2, 0, 128).rearrange("p i (y x) -> p i y x", y=H)
                        nc.vector.tensor_tensor(zo, hi, ci, OP.mult)
                        z_t = dBx
                    else:
                        nc.vector.tensor_tensor(gv(t1, 0, 128), gv(t1, 0, 128),
                                                gv(t2, 0, 128), OP.mult)
                        z_t = t1
                    # out-proj accumulation for this group
                    for t in range(T):
                        off = t * 512
                        if g == 0:
                            nc.tensor.matmul(ps_y[:], kt['opw'][:, 0:64],
                                             z_t[:, off:off + 512],
                                             start=True, stop=True)
                            nc.scalar.activation(y_t[:, off:off + 512],
                                                 ps_y[:], AF.Copy)
                        elif g < 3:
                            nc.tensor.matmul(ps_y[:],
                                             kt['opw'][:, g * 64:(g + 1) * 64],
                                             z_t[:, off:off + 512],
                                             start=True, stop=True)
                            nc.vector.scalar_tensor_tensor(
                                y_t[:, off:off + 512], ps_y[:], 1.0,
                                y_t[:, off:off + 512], OP.mult, OP.add)
                        else:
                            nc.tensor.matmul(ps_y[:],
                                             kt['opw'][:, g * 64:(g + 1) * 64],
                                             z_t[:, off:off + 512],
                                             start=True, stop=False)
                            nc.tensor.matmul(ps_y[:], kt['dpw'][:],
                                             xnc[:, off:off + 512],
                                             start=False, stop=True)
                            nc.vector.scalar_tensor_tensor(
                                y_t[:, off:off + 512], ps_y[:], kt['outb'][:],
                                y_t[:, off:off + 512], OP.add, OP.add)

                # ---- CA gate ----
                yv = y_t[:, 0:F].rearrange("p (i c) -> p i c", i=IPC)[:, :, 0:L]
                with nc.allow_low_precision(reason="ymean bf16 output"):
                    nc.vector.tensor_reduce(ymean[:, 0:1], yv[:, 0, :],
                                            mybir.AxisListType.X, OP.add)
                    nc.vector.tensor_reduce(ymean[:, 1:2], yv[:, 1, :],
                                            mybir.AxisListType.X, OP.add)
                nc.tensor.matmul(ps_ca1[:], kt['ca1w'][:], ymean[:],
                                 start=True, stop=True)
                nc.scalar.activation(ca1s[:], ps_ca1[:], AF.Relu,
                                     bias=kt['ca1b'][:])
                nc.tensor.matmul(ps_ca2[:], kt['ca2w'][:], ca1s[:],
                                 start=True, stop=True)
                nc.scalar.activation(ca[:], ps_ca2[:], AF.Exp, scale=-1.0,
                                     bias=kt['ca2bn'][:])
                nc.vector.tensor_scalar_add(ca[:], ca[:], 1.0)
                nc.vector.reciprocal(ca[:], ca[:])

                # ---- final: res = y*ca + (x + oc + depb); cast to f32 on DMA
                xv = gv(xx2, 0, 64)
                for i in range(IPC):
                    nc.vector.scalar_tensor_tensor(
                        yv[:, i, :], yv[:, i, :], ca[:, i:i + 1], xv[:, i, :],
                        OP.mult, OP.add)
                nc.gpsimd.dma_start(
                    out_f[:, :].rearrange("(i c) l -> c i l", i=IPC), yv)

    return nc


def kernel(__reps=1, **inputs):
    inputs = {k: np.asarray(v) for k, v in inputs.items()}
    x = inputs['x'].astype(np.float32)
    key = f"v2r{__reps}"
    if key not in _CACHE:
        _CACHE[key] = _build(__reps)
    nc = _CACHE[key]
    consts = _make_consts(inputs)
    in_maps = []
    for core in range(NCORES):
        m = {'x': np.ascontiguousarray(x[core * IPC:(core + 1) * IPC])}
        for name, _, _ in CONST_SPECS:
            m[name] = np.ascontiguousarray(consts[name].astype(np.float32))
        in_maps.append(m)
    res = run_bass_kernel_spmd(nc, in_maps, list(range(NCORES)))
    outs = [res.results[i]['out'] for i in range(NCORES)]
    return np.concatenate(outs, axis=0).astype(np.float32)
